# revision 88
# baseline (speedup 1.0000x reference)
# Causal self-attention on 8 TRN2 NeuronCores.
#
# Sharding (data + tensor parallel per the hint):
#   core c -> batch b = c // 4, head group g = c % 4 (4 heads of 64 dims = 256).
#   Wq/Wk/Wv are split column-wise (rows of W, since y = x @ W.T) per head
#   group; Wo is split row-wise. Each core computes a partial [S, D] output
#   (transposed on device as [D, S]); the host sums the 4 partials per batch
#   element (the "all-reduce" of row-parallel sharding) and transposes back.
#
# Device kernel (per core), all matmuls in fp32r (full-rate PE):
#   xT [D, S] resident in SBUF.
#   QT/KT [d'=256, S] = W x   (head dim on partitions; 1/8 scale folded
#                              into Wq/bq on the host; bq added on DVE during
#                              the PSUM->SBUF copy; bk dropped exactly --
#                              softmax is shift-invariant per query row; bv
#                              folded into bo on the host -- attn rows sum 1)
#   V    [S, d'=256]          (sequence on partitions)
#   per head pair (row-packed K=64 matmuls) and q-block of 512:
#     scoresT [k,q] = KT.T-free matmul;
#     exp on ACT (no max subtraction -- scores are O(+-8), safe in fp32);
#     causal: skip fully-masked k-chunks, mask the 128x128 diagonal triangle
#     on DVE (off the critical path thanks to the lag-2 PV emission);
#     PV accumulates [O; rowsum] over k-chunks via a ones-augmented V, and is
#     emitted with a lag of 2 chunks behind the score matmuls so the in-order
#     PE stream never stalls waiting for the ACT exp of the current chunk;
#     normalize via K=1 PE broadcast of the sums + DVE approx reciprocal
#     (gpsimd partition_broadcast is broken on HW; DVE is lane-aligned).
#   partialT [D, S] = WoT.T-free matmul over d' chunks, + bo (only on g==0
#   cores), DMA'd out.

import os

import numpy as np

S = 2048
D = 1024
DL = 256  # local head dims (4 heads x 64)
NCORES = 8

_cache = {}
LAST_EXEC_TIME_NS = None
LAST_TRACE_PATH = None


DEBUG = os.environ.get("KERNEL_DEBUG", "0") == "1"


def _build_bass():
    from concourse import bacc
    import concourse.tile as tile
    import concourse.mybir as mybir
    from concourse.bass import ts, ds

    f32 = mybir.dt.float32
    f32r = mybir.dt.float32r
    bf16 = mybir.dt.bfloat16
    f16 = mybir.dt.float16
    Exp = mybir.ActivationFunctionType.Exp
    Ident = mybir.ActivationFunctionType.Identity
    ADD = mybir.AluOpType.add

    nc = bacc.Bacc("TRN2", target_bir_lowering=False, debug=False)

    # weights arrive pre-arranged [partition, chunk*free] so their DMAs are
    # contiguous 4KB-line copies (the rearranged-AP version generated 512B
    # descriptor lines and ran ~3x slower, throttling the startup queues)
    xT_d = nc.dram_tensor("xT", [D, S], bf16, kind="ExternalInput")
    wqT_d = nc.dram_tensor("wqT", [128, 8 * DL], bf16, kind="ExternalInput")
    wkT_d = nc.dram_tensor("wkT", [128, 8 * DL], bf16, kind="ExternalInput")
    wvT_d = nc.dram_tensor("wvT", [128, 8 * DL], bf16, kind="ExternalInput")
    woT_d = nc.dram_tensor("woT", [128, 2 * D], bf16, kind="ExternalInput")
    woT2_d = nc.dram_tensor("woT2", [64, D], bf16, kind="ExternalInput")
    bq_d = nc.dram_tensor("bq", [128, 2], f32r, kind="ExternalInput")
    bo_d = nc.dram_tensor("bo", [128, 8], f32, kind="ExternalInput")
    mask_d = nc.dram_tensor("mask", [128, 128], bf16, kind="ExternalInput")
    onesr_d = nc.dram_tensor("onesr", [128, 64], f32r, kind="ExternalInput")
    out_d = nc.dram_tensor("outT", [D, S], bf16, kind="ExternalOutput")
    out2_d = nc.dram_tensor("outT2", [D, 512], bf16, kind="ExternalOutput")
    warm_d = nc.dram_tensor("warm", [1, 512], f32, kind="ExternalOutput")
    if DEBUG:
        qT_o = nc.dram_tensor("qT_o", [128, 2, S], f32r, kind="ExternalOutput")
        kT_o = nc.dram_tensor("kT_o", [128, 2, S], f32r, kind="ExternalOutput")
        v4_o = nc.dram_tensor("v4_o", [128, 16, 4, 65], f32r, kind="ExternalOutput")
        oT_o = nc.dram_tensor("oT_o", [128, 2, S], f32r, kind="ExternalOutput")

    with tile.TileContext(nc) as tc:
        with (
            tc.tile_pool(name="persist", bufs=1) as persist,
            tc.tile_pool(name="ptp", bufs=4) as ptp,
            tc.tile_pool(name="sup", bufs=2) as sup,
            tc.tile_pool(name="oup", bufs=2) as oup,
            tc.tile_pool(name="rbp", bufs=2) as rbp,
            tc.tile_pool(name="stp", bufs=2) as stp,
            tc.tile_pool(name="tbp", bufs=1) as tbp,
            tc.tile_pool(name="sc2", bufs=2, space="PSUM") as sc2,
            tc.tile_pool(name="mm", bufs=2, space="PSUM") as mm,
            tc.tile_pool(name="po", bufs=2, space="PSUM") as po,
        ):
            # ---- persistent SBUF tensors ----
            xT = persist.tile([128, 8, S], bf16, name="xT_sb")
            wqT = persist.tile([128, 8, DL], bf16, name="wqT_sb")
            wkT = persist.tile([128, 8, DL], bf16, name="wkT_sb")
            wvT = persist.tile([128, 8, DL], bf16, name="wvT_sb")
            woT = persist.tile([128, 2, D], bf16, name="woT_sb")
            woT2 = persist.tile([64, D], bf16, name="woT2_sb")
            bq = persist.tile([128, 2], f32r, name="bq_sb")
            bo = persist.tile([128, 8], f32, name="bo_sb")
            mask = persist.tile([128, 128], bf16, name="mask_sb")
            ones = persist.tile([128, 64], f32r, name="ones_sb")
            ones_bf = persist.tile([128, 512], bf16, name="ones_bf")
            qT = persist.tile([128, 2, S], f16, name="qT_sb")
            kT = persist.tile([128, 2, S], f16, name="kT_sb")
            v4 = persist.tile([128, 16, 4, 65], bf16, name="v4_sb")
            oT = persist.tile([128, 2, S], bf16, name="oT_sb")

            # ---- input DMAs: first-wave x + weights first, both HW queues ----
            x_r = xT_d.ap().rearrange("(o p) f -> p o f", p=128)
            nc.vector.memset(ones_bf[:], 1.0)
            for mc in range(8):
                eng = nc.sync if mc % 2 == 0 else nc.scalar
                eng.dma_start(xT[:, mc, ts(0, 1024)], x_r[:, mc, ts(0, 1024)])
            nc.sync.dma_start(wqT[:], wqT_d.ap())
            nc.scalar.dma_start(wkT[:], wkT_d.ap())
            nc.sync.dma_start(bq[:], bq_d.ap())
            nc.scalar.dma_start(mask[:], mask_d.ap())
            nc.sync.dma_start(ones[:], onesr_d.ap())
            nc.scalar.dma_start(v4[:, :, :, 64:65], ones_bf[:, 0:64])
            nc.scalar.dma_start(wvT[:], wvT_d.ap())
            for mc in range(8):
                eng = nc.sync if mc % 2 == 0 else nc.scalar
                eng.dma_start(xT[:, mc, ts(1, 1024)], x_r[:, mc, ts(1, 1024)])
            nc.sync.dma_start(woT[:], woT_d.ap())
            nc.scalar.dma_start(woT2[:], woT2_d.ap())
            nc.sync.dma_start(bo[:], bo_d.ap())

            # PE warmup: enough back-to-back matmuls to ramp the tensor
            # engine to its top p-state (~3us continuous) before real work.
            psW = sc2.tile([128, 2, 512], f32, tag="sc", name="psW")
            for i in range(24):
                nc.tensor.matmul(
                    psW[:, i % 2, :],
                    lhsT=ones_bf[:, 0:128],
                    rhs=ones_bf[:],
                    start=True,
                    stop=True,
                    skip_group_check=True,
                )
            wstg = stp.tile([1, 512], f32, tag="wst", name="wstg", bufs=1)
            nc.vector.tensor_copy(wstg[:], psW[0:1, 0, :])
            nc.sync.dma_start(warm_d.ap(), wstg[:])

            def proj_qk(wsb, dst, t, qb, bias=None):
                ps = mm.tile([128, 512], f32, tag="mm")
                for mc in range(8):
                    nc.tensor.matmul(
                        ps,
                        lhsT=wsb[:, mc, ts(t, 128)],
                        rhs=xT[:, mc, ts(qb, 512)],
                        start=(mc == 0),
                        stop=(mc == 7),
                    )
                if bias is not None:
                    nc.vector.tensor_tensor(
                        dst[:, t, ts(qb, 512)],
                        ps,
                        bias[:, t : t + 1].to_broadcast((128, 512)),
                        ADD,
                    )
                else:
                    nc.vector.tensor_copy(dst[:, t, ts(qb, 512)], ps)

            def proj_v(st):
                ps = mm.tile([128, 512], f32, tag="mm")
                psv = ps[:, 0:256]
                for mc in range(8):
                    nc.tensor.matmul(
                        psv,
                        lhsT=xT[:, mc, ts(st, 128)],
                        rhs=wvT[:, mc, :],
                        start=(mc == 0),
                        stop=(mc == 7),
                    )
                nc.vector.tensor_copy(
                    v4[:, st, :, 0:64], psv.rearrange("p (h d) -> p h d", h=4)
                )

            state = {}

            def attn_block(
                pair, qb, fill=None, fill_every=1, fill_offset=0, prefill=None
            ):
                # heads (2*pair, 2*pair+1); q columns [512*qb, 512*qb+512)
                psA = po.tile([128, 512], f32, tag="po")
                psB = po.tile([128, 512], f32, tag="po")
                nchunks = 4 * qb + 4

                def emit_pv(ent, last):
                    pt_, c_, q0_, w_ = ent
                    first = c_ == 0
                    for hh, psO in ((0, psA), (1, psB)):
                        nc.tensor.matmul(
                            psO[0:65, ds(q0_, w_)],
                            lhsT=v4[:, c_, 2 * pair + hh, :],
                            rhs=pt_[:, hh, :w_],
                            start=first,
                            stop=last,
                            skip_group_check=True,
                        )

                pend = []
                for c in range(nchunks):
                    if prefill and c == 1:
                        # previous block's deferred normalization: runs here
                        # so the PE never stalls on it at the block boundary
                        for p in prefill:
                            p()
                        prefill = None
                    if (
                        fill
                        and c >= fill_offset
                        and (c - fill_offset) % fill_every == fill_every - 1
                    ):
                        fill.pop(0)()
                    dc = c - 4 * qb
                    q0 = 128 * dc if dc >= 0 else 0
                    w = 512 - q0
                    ps2 = sc2.tile([128, 2, 512], f32, tag="sc")
                    for hh in (0, 1):
                        prow = slice(64 * hh, 64 * hh + 64)
                        nc.tensor.matmul(
                            ps2[:, hh, :w],
                            lhsT=kT[prow, pair, ts(c, 128)],
                            rhs=qT[prow, pair, ds(512 * qb + q0, w)],
                            start=True,
                            stop=True,
                        )
                    pt = ptp.tile([128, 2, 512], bf16, tag="pt")
                    nc.scalar.activation(pt[:, :, :w], ps2[:, :, :w], Exp)
                    if dc >= 0:
                        # zero the masked upper triangle of the diagonal
                        # 128x128 tile on the (otherwise idle) gpsimd
                        # engine; the lag-2 PV emission below gives it ~2
                        # chunk-slots of slack off the critical path
                        nc.gpsimd.tensor_mul(
                            pt[:, :, 0:128],
                            pt[:, :, 0:128],
                            mask[:, None, :].to_broadcast((128, 2, 128)),
                        )
                    pend.append((pt, c, q0, w))
                    if len(pend) > 3:
                        emit_pv(pend.pop(0), last=False)
                while pend:
                    ent = pend.pop(0)
                    emit_pv(ent, last=(not pend))

                # normalization, returned as a closure the caller defers
                # into the NEXT block (emitted at its chunk 1): one fused
                # [65,512] copy per head frees the PSUM accumulators early,
                # the PE broadcasts the rowsums, DVE takes the reciprocals,
                # and the (PSUM-free) multiplies run on gpsimd
                def do_norm():
                    oUA = oup.tile([65, 512], f32r, tag="ou")
                    oUB = oup.tile([65, 512], f32r, tag="ou")
                    nc.vector.tensor_copy(oUA[:, :], psA[0:65, :])
                    nc.vector.tensor_copy(oUB[:, :], psB[0:65, :])
                    psR = mm.tile([128, 512], f32, tag="mm")
                    nc.tensor.matmul(
                        psR[0:64, :],
                        lhsT=ones[64:65, 0:64],
                        rhs=oUA[64:65, :],
                        start=True,
                        stop=True,
                    )
                    psR2 = mm.tile([128, 512], f32, tag="mm")
                    nc.tensor.matmul(
                        psR2[0:64, :],
                        lhsT=ones[64:65, 0:64],
                        rhs=oUB[64:65, :],
                        start=True,
                        stop=True,
                    )
                    rbA = rbp.tile([64, 512], f32, tag="rb")
                    rbB = rbp.tile([64, 512], f32, tag="rb")
                    nc.vector.reciprocal_approx_fast(rbA[:, :], psR[0:64, :])
                    nc.vector.reciprocal_approx_fast(rbB[:, :], psR2[0:64, :])
                    tmpB = tbp.tile([64, 512], bf16, tag="tb")
                    nc.gpsimd.tensor_mul(
                        oT[0:64, pair, ts(qb, 512)], oUA[0:64, :], rbA[:, :]
                    )
                    nc.gpsimd.tensor_mul(tmpB[:, :], oUB[0:64, :], rbB[:, :])
                    if pair == 1 and qb == 3:
                        # last block: out_proj_B contracts tmpB directly
                        # (via woT2) -- no partition-shift DMA on the tail
                        state["tmpB3"] = tmpB
                    else:
                        nc.sync.dma_start(
                            oT[64:128, pair, ts(qb, 512)], tmpB[:, :]
                        )

                return do_norm

            def out_proj_jt(jt, sb):
                    ps = mm.tile([128, 512], f32, tag="mm")
                    for dchunk in range(2):
                        nc.tensor.matmul(
                            ps,
                            lhsT=woT[:, dchunk, ts(jt, 128)],
                            rhs=oT[:, dchunk, ts(sb, 512)],
                            start=(dchunk == 0),
                            stop=(dchunk == 1),
                        )
                    stg = stp.tile([128, 512], bf16, tag="st")
                    nc.vector.tensor_tensor(
                        stg[:],
                        ps,
                        bo[:, jt : jt + 1].to_broadcast((128, 512)),
                        ADD,
                    )
                    nc.sync.dma_start(out_d.ap()[ts(jt, 128), ts(sb, 512)], stg[:])

            def out_proj_A_jt(jt):
                # pair-0 half of the sb=3 output projection (+ bias); runs
                # as filler inside the final attention block
                ps = mm.tile([128, 512], f32, tag="mm")
                nc.tensor.matmul(
                    ps,
                    lhsT=woT[:, 0, ts(jt, 128)],
                    rhs=oT[:, 0, ts(3, 512)],
                    start=True,
                    stop=True,
                )
                stg = stp.tile([128, 512], bf16, tag="st")
                nc.vector.tensor_tensor(
                    stg[:],
                    ps,
                    bo[:, jt : jt + 1].to_broadcast((128, 512)),
                    ADD,
                )
                nc.sync.dma_start(out_d.ap()[ts(jt, 128), ts(3, 512)], stg[:])

            def out_proj_B_jt(jt):
                # pair-1 half, shipped as a separate bf16 partial (host
                # adds); head-3 dims contracted straight out of tmpB3 (no
                # partition-shift DMA); PSUM->SBUF copy on the tail-idle
                # ACT engine
                ps = mm.tile([128, 512], f32, tag="mm")
                nc.tensor.matmul(
                    ps,
                    lhsT=woT[0:64, 1, ts(jt, 128)],
                    rhs=oT[0:64, 1, ts(3, 512)],
                    start=True,
                    stop=False,
                )
                nc.tensor.matmul(
                    ps,
                    lhsT=woT2[:, ts(jt, 128)],
                    rhs=state["tmpB3"][:, :],
                    start=False,
                    stop=True,
                )
                stg = stp.tile([128, 512], bf16, tag="st2", bufs=4)
                nc.scalar.activation(stg[:], ps, Ident)
                eng = nc.sync if jt % 2 == 0 else nc.scalar
                eng.dma_start(out2_d.ap()[ts(jt, 128), :], stg[:])

            # software-pipelined emission: per q-block wave, produce the
            # projections it needs, then attention, then the output slice
            def emit_A(qb):
                for t in range(2):
                    proj_qk(wqT, qT, t, qb, bias=bq)
                for st in range(4 * qb, 4 * qb + 4):
                    proj_v(st)
                for t in range(2):
                    proj_qk(wkT, kT, t, qb)

            emit_A(0)
            for qb in range(4):
                ath = []
                if qb < 3:
                    nxt = qb + 1
                    for t in range(2):
                        ath.append(
                            lambda t=t, nxt=nxt: proj_qk(wqT, qT, t, nxt, bias=bq)
                        )
                    for st in range(4 * nxt, 4 * nxt + 4):
                        ath.append(lambda st=st: proj_v(st))
                    for t in range(2):
                        ath.append(
                            lambda t=t, nxt=nxt: proj_qk(wkT, kT, t, nxt)
                        )
                cth = []
                if qb == 1:
                    cth = [
                        lambda jt=jt: out_proj_jt(jt, 0) for jt in range(8)
                    ]
                elif qb == 2:
                    cth = [
                        lambda jt=jt: out_proj_jt(jt, 1) for jt in range(4)
                    ]
                elif qb == 3:
                    cth = [
                        lambda jt=jt: out_proj_jt(jt + 4, 1) for jt in range(4)
                    ] + [
                        lambda jt=jt: out_proj_jt(jt, 2) for jt in range(8)
                    ]
                # projections (dependency-free) fill first; out-projections
                # need the previous wave's deferred normalize, so they go
                # later in the wave
                thunks = ath + cth
                pn = state.get("prev_norm")
                pre = [pn] if pn else None
                if qb < 3:
                    # ceil so fills spread over BOTH pair blocks (floor
                    # front-loads them into pair 0 and starves pair 1,
                    # tripping the HAM half-throttle)
                    fe = max(1, -((-2 * (4 * qb + 4)) // (len(thunks) + 1)))
                    n0 = attn_block(
                        0, qb, fill=thunks, fill_every=fe, prefill=pre
                    )
                    n1 = attn_block(
                        1, qb, fill=thunks, fill_every=fe, prefill=[n0]
                    )
                    state["prev_norm"] = n1
                    for th in thunks:
                        th()
                else:
                    # final wave: pair-0 block takes the shared thunks;
                    # pair-1 block overlaps the pair-0 half of the sb=3
                    # output projection (fill_offset leaves time for the
                    # deferred pair-0 normalize to land)
                    # fill_every=2 leaves ~5 thunks for the pair-1 block so
                    # the ACT-bound tail keeps enough PE density to hold
                    # the HAM governor at full throughput
                    n0 = attn_block(
                        0, qb, fill=thunks, fill_every=2,
                        fill_offset=2, prefill=pre,
                    )
                    thB = thunks + [
                        lambda jt=jt: out_proj_A_jt(jt) for jt in range(8)
                    ]
                    feB = max(1, (4 * qb + 4 - 8) // (len(thB) + 1))
                    n1 = attn_block(
                        1, qb, fill=thB, fill_every=feB,
                        fill_offset=8, prefill=[n0],
                    )
                    for th in thB:
                        th()
                    n1()
                    for jt in range(8):
                        out_proj_B_jt(jt)
            if DEBUG:
                nc.sync.dma_start(qT_o.ap(), qT[:])
                nc.sync.dma_start(kT_o.ap(), kT[:])
                nc.sync.dma_start(v4_o.ap(), v4[:])
                nc.sync.dma_start(oT_o.ap(), oT[:])

    nc.compile()
    return nc


def _get_bass():
    if "nc" not in _cache:
        _cache["nc"] = _build_bass()
    return _cache["nc"]


def _shard_inputs(x, Wq, bq, Wk, bk, Wv, bv, Wo, bo):
    import ml_dtypes

    bfl = ml_dtypes.bfloat16
    x = np.asarray(x, dtype=np.float32)
    Wq = np.asarray(Wq, dtype=np.float32)
    Wk = np.asarray(Wk, dtype=np.float32)
    Wv = np.asarray(Wv, dtype=np.float32)
    Wo = np.asarray(Wo, dtype=np.float32)
    bq = np.asarray(bq, dtype=np.float32)
    bv = np.asarray(bv, dtype=np.float32)
    bo = np.asarray(bo, dtype=np.float32)

    kk = np.arange(128)[:, None]
    qq = np.arange(128)[None, :]
    mask128 = (kk <= qq).astype(bfl)

    xT = [np.ascontiguousarray(x[b].T).astype(bfl) for b in range(x.shape[0])]

    def prearrange(wT, nchunks):
        # [nchunks*128, F] -> [128, nchunks*F] so each partition's row is
        # one contiguous DMA line
        f = wT.shape[1]
        return np.ascontiguousarray(
            wT.reshape(nchunks, 128, f).transpose(1, 0, 2).reshape(128, -1)
        )
    in_maps = []
    for c in range(NCORES):
        b, g = divmod(c, 4)
        sl = slice(DL * g, DL * (g + 1))
        # bv contributes a constant row to this core's partial output
        # (attention rows sum to 1); bo itself only on the g==0 cores.
        bo_eff = bv[sl] @ Wo[:, sl].T
        if g == 0:
            bo_eff = bo_eff + bo
        in_maps.append(
            {
                "xT": xT[b],
                "wqT": prearrange(Wq[sl].T * 0.125, 8).astype(bfl),
                "wkT": prearrange(Wk[sl].T, 8).astype(bfl),
                "wvT": prearrange(Wv[sl].T, 8).astype(bfl),
                "woT": prearrange(Wo[:, sl].T, 2).astype(bfl),
                "woT2": np.ascontiguousarray(Wo[:, sl].T[192:256]).astype(bfl),
                "bq": np.ascontiguousarray((bq[sl] * 0.125).reshape(2, 128).T),
                "bo": np.ascontiguousarray(bo_eff.reshape(8, 128).T),
                "mask": mask128,
                "onesr": np.ones((128, 64), np.float32),
            }
        )
    return in_maps


def kernel(x, Wq, bq, Wk, bk, Wv, bv, Wo, bo):
    global LAST_EXEC_TIME_NS, LAST_TRACE_PATH
    from concourse.bass_utils import run_bass_kernel_spmd

    nc = _get_bass()
    in_maps = _shard_inputs(x, Wq, bq, Wk, bk, Wv, bv, Wo, bo)

    trace = os.environ.get("KERNEL_TRACE", "0") == "1"
    res = run_bass_kernel_spmd(
        nc, in_maps, core_ids=list(range(NCORES)), trace=trace
    )
    LAST_EXEC_TIME_NS = res.exec_time_ns
    if res.instructions_and_trace is not None:
        LAST_TRACE_PATH = res.instructions_and_trace[1]

    B = 2
    out = np.empty((B, S, D), dtype=np.float32)
    for b in range(B):
        acc = res.results[4 * b]["outT"].astype(np.float32)
        acc[:, 1536:2048] += res.results[4 * b]["outT2"].astype(np.float32)
        for g in range(1, 4):
            acc = acc + res.results[4 * b + g]["outT"].astype(np.float32)
            acc[:, 1536:2048] += res.results[4 * b + g]["outT2"].astype(
                np.float32
            )
        out[b] = acc.T
    return out


# revision 90
# speedup vs baseline: 1.0402x; 1.0402x over previous
# Causal self-attention on 8 TRN2 NeuronCores.
#
# Sharding (data + tensor parallel per the hint):
#   core c -> batch b = c // 4, head group g = c % 4 (4 heads of 64 dims = 256).
#   Wq/Wk/Wv are split column-wise (rows of W, since y = x @ W.T) per head
#   group; Wo is split row-wise. Each core computes a partial [S, D] output
#   (transposed on device as [D, S]); the host sums the 4 partials per batch
#   element (the "all-reduce" of row-parallel sharding) and transposes back.
#
# Device kernel (per core), all matmuls in fp32r (full-rate PE):
#   xT [D, S] resident in SBUF.
#   QT/KT [d'=256, S] = W x   (head dim on partitions; 1/8 scale folded
#                              into Wq/bq on the host; bq added on DVE during
#                              the PSUM->SBUF copy; bk dropped exactly --
#                              softmax is shift-invariant per query row; bv
#                              folded into bo on the host -- attn rows sum 1)
#   V    [S, d'=256]          (sequence on partitions)
#   per head pair (row-packed K=64 matmuls) and q-block of 512:
#     scoresT [k,q] = KT.T-free matmul;
#     exp on ACT (no max subtraction -- scores are O(+-8), safe in fp32);
#     causal: skip fully-masked k-chunks, mask the 128x128 diagonal triangle
#     on DVE (off the critical path thanks to the lag-2 PV emission);
#     PV accumulates [O; rowsum] over k-chunks via a ones-augmented V, and is
#     emitted with a lag of 2 chunks behind the score matmuls so the in-order
#     PE stream never stalls waiting for the ACT exp of the current chunk;
#     normalize via K=1 PE broadcast of the sums + DVE approx reciprocal
#     (gpsimd partition_broadcast is broken on HW; DVE is lane-aligned).
#   partialT [D, S] = WoT.T-free matmul over d' chunks, + bo (only on g==0
#   cores), DMA'd out.

import os

import numpy as np

S = 2048
D = 1024
DL = 256  # local head dims (4 heads x 64)
NCORES = 8

_cache = {}
LAST_EXEC_TIME_NS = None
LAST_TRACE_PATH = None


DEBUG = os.environ.get("KERNEL_DEBUG", "0") == "1"


def _build_bass():
    from concourse import bacc
    import concourse.tile as tile
    import concourse.mybir as mybir
    from concourse.bass import ts, ds

    f32 = mybir.dt.float32
    f32r = mybir.dt.float32r
    bf16 = mybir.dt.bfloat16
    f16 = mybir.dt.float16
    Exp = mybir.ActivationFunctionType.Exp
    Ident = mybir.ActivationFunctionType.Identity
    ADD = mybir.AluOpType.add

    nc = bacc.Bacc("TRN2", target_bir_lowering=False, debug=False)

    # weights arrive pre-arranged [partition, chunk*free] so their DMAs are
    # contiguous 4KB-line copies (the rearranged-AP version generated 512B
    # descriptor lines and ran ~3x slower, throttling the startup queues)
    xT_d = nc.dram_tensor("xT", [D, S], bf16, kind="ExternalInput")
    wqT_d = nc.dram_tensor("wqT", [128, 8 * DL], bf16, kind="ExternalInput")
    wkT_d = nc.dram_tensor("wkT", [128, 8 * DL], bf16, kind="ExternalInput")
    wvT_d = nc.dram_tensor("wvT", [128, 8 * DL], bf16, kind="ExternalInput")
    woT_d = nc.dram_tensor("woT", [128, 2 * D], bf16, kind="ExternalInput")
    woT2_d = nc.dram_tensor("woT2", [64, D], bf16, kind="ExternalInput")
    bq_d = nc.dram_tensor("bq", [128, 2], f32r, kind="ExternalInput")
    bo_d = nc.dram_tensor("bo", [128, 8], f32, kind="ExternalInput")
    mask_d = nc.dram_tensor("mask", [128, 128], bf16, kind="ExternalInput")
    onesr_d = nc.dram_tensor("onesr", [128, 64], f32r, kind="ExternalInput")
    out_d = nc.dram_tensor("outT", [D, S], bf16, kind="ExternalOutput")
    out2_d = nc.dram_tensor("outT2", [D, 512], bf16, kind="ExternalOutput")
    warm_d = nc.dram_tensor("warm", [1, 512], f32, kind="ExternalOutput")
    if DEBUG:
        qT_o = nc.dram_tensor("qT_o", [128, 2, S], f32r, kind="ExternalOutput")
        kT_o = nc.dram_tensor("kT_o", [128, 2, S], f32r, kind="ExternalOutput")
        v4_o = nc.dram_tensor("v4_o", [128, 16, 4, 65], f32r, kind="ExternalOutput")
        oT_o = nc.dram_tensor("oT_o", [128, 2, S], f32r, kind="ExternalOutput")

    with tile.TileContext(nc) as tc:
        with (
            tc.tile_pool(name="persist", bufs=1) as persist,
            tc.tile_pool(name="ptp", bufs=4) as ptp,
            tc.tile_pool(name="sup", bufs=2) as sup,
            tc.tile_pool(name="oup", bufs=2) as oup,
            tc.tile_pool(name="rbp", bufs=2) as rbp,
            tc.tile_pool(name="stp", bufs=2) as stp,
            tc.tile_pool(name="tbp", bufs=1) as tbp,
            tc.tile_pool(name="sc2", bufs=2, space="PSUM") as sc2,
            tc.tile_pool(name="mm", bufs=2, space="PSUM") as mm,
            tc.tile_pool(name="po", bufs=2, space="PSUM") as po,
        ):
            # ---- persistent SBUF tensors ----
            xT = persist.tile([128, 8, S], bf16, name="xT_sb")
            wqT = persist.tile([128, 8, DL], bf16, name="wqT_sb")
            wkT = persist.tile([128, 8, DL], bf16, name="wkT_sb")
            wvT = persist.tile([128, 8, DL], bf16, name="wvT_sb")
            woT = persist.tile([128, 2, D], bf16, name="woT_sb")
            woT2 = persist.tile([64, D], bf16, name="woT2_sb")
            bq = persist.tile([128, 2], f32r, name="bq_sb")
            bo = persist.tile([128, 8], f32, name="bo_sb")
            mask = persist.tile([128, 128], bf16, name="mask_sb")
            ones = persist.tile([128, 64], f32r, name="ones_sb")
            ones_bf = persist.tile([128, 512], bf16, name="ones_bf")
            qT = persist.tile([128, 2, S], f16, name="qT_sb")
            kT = persist.tile([128, 2, S], f16, name="kT_sb")
            v4 = persist.tile([128, 16, 4, 65], bf16, name="v4_sb")
            oT = persist.tile([128, 2, S], bf16, name="oT_sb")

            # ---- input DMAs: first-wave x + weights first, both HW queues ----
            x_r = xT_d.ap().rearrange("(o p) f -> p o f", p=128)
            nc.vector.memset(ones_bf[:], 1.0)
            for mc in range(8):
                eng = nc.sync if mc % 2 == 0 else nc.scalar
                eng.dma_start(xT[:, mc, ts(0, 1024)], x_r[:, mc, ts(0, 1024)])
            nc.sync.dma_start(wqT[:], wqT_d.ap())
            nc.scalar.dma_start(wkT[:], wkT_d.ap())
            nc.sync.dma_start(bq[:], bq_d.ap())
            nc.scalar.dma_start(mask[:], mask_d.ap())
            nc.sync.dma_start(ones[:], onesr_d.ap())
            nc.scalar.dma_start(v4[:, :, :, 64:65], ones_bf[:, 0:64])
            nc.scalar.dma_start(wvT[:], wvT_d.ap())
            for mc in range(8):
                eng = nc.sync if mc % 2 == 0 else nc.scalar
                eng.dma_start(xT[:, mc, ts(1, 1024)], x_r[:, mc, ts(1, 1024)])
            nc.sync.dma_start(woT[:], woT_d.ap())
            nc.scalar.dma_start(woT2[:], woT2_d.ap())
            nc.sync.dma_start(bo[:], bo_d.ap())

            # PE warmup: enough back-to-back matmuls to ramp the tensor
            # engine to its top p-state (~3us continuous) before real work.
            psW = sc2.tile([128, 2, 512], f32, tag="sc", name="psW")
            for i in range(24):
                nc.tensor.matmul(
                    psW[:, i % 2, :],
                    lhsT=ones_bf[:, 0:128],
                    rhs=ones_bf[:],
                    start=True,
                    stop=True,
                    skip_group_check=True,
                )
            wstg = stp.tile([1, 512], f32, tag="wst", name="wstg", bufs=1)
            nc.vector.tensor_copy(wstg[:], psW[0:1, 0, :])
            nc.sync.dma_start(warm_d.ap(), wstg[:])

            def proj_qk(wsb, dst, t, qb, bias=None):
                ps = mm.tile([128, 512], f32, tag="mm")
                for mc in range(8):
                    nc.tensor.matmul(
                        ps,
                        lhsT=wsb[:, mc, ts(t, 128)],
                        rhs=xT[:, mc, ts(qb, 512)],
                        start=(mc == 0),
                        stop=(mc == 7),
                    )
                if bias is not None:
                    nc.vector.tensor_tensor(
                        dst[:, t, ts(qb, 512)],
                        ps,
                        bias[:, t : t + 1].to_broadcast((128, 512)),
                        ADD,
                    )
                else:
                    nc.vector.tensor_copy(dst[:, t, ts(qb, 512)], ps)

            def proj_v(st):
                ps = mm.tile([128, 512], f32, tag="mm")
                psv = ps[:, 0:256]
                for mc in range(8):
                    nc.tensor.matmul(
                        psv,
                        lhsT=xT[:, mc, ts(st, 128)],
                        rhs=wvT[:, mc, :],
                        start=(mc == 0),
                        stop=(mc == 7),
                    )
                nc.vector.tensor_copy(
                    v4[:, st, :, 0:64], psv.rearrange("p (h d) -> p h d", h=4)
                )

            state = {}

            def attn_block(
                pair, qb, fill=None, fill_every=1, fill_offset=0, prefill=None
            ):
                # heads (2*pair, 2*pair+1); q columns [512*qb, 512*qb+512)
                psA = po.tile([128, 512], f32, tag="po")
                psB = po.tile([128, 512], f32, tag="po")
                nchunks = 4 * qb + 4

                def emit_pv(ent, last):
                    pt_, c_, q0_, w_ = ent
                    first = c_ == 0
                    for hh, psO in ((0, psA), (1, psB)):
                        nc.tensor.matmul(
                            psO[0:65, ds(q0_, w_)],
                            lhsT=v4[:, c_, 2 * pair + hh, :],
                            rhs=pt_[:, hh, :w_],
                            start=first,
                            stop=last,
                            skip_group_check=True,
                        )

                pend = []
                for c in range(nchunks):
                    if prefill and c == 1:
                        # previous block's deferred normalization: runs here
                        # so the PE never stalls on it at the block boundary
                        for p in prefill:
                            p()
                        prefill = None
                    if (
                        fill
                        and c >= fill_offset
                        and (c - fill_offset) % fill_every == fill_every - 1
                    ):
                        fill.pop(0)()
                    dc = c - 4 * qb
                    q0 = 128 * dc if dc >= 0 else 0
                    w = 512 - q0
                    ps2 = sc2.tile([128, 2, 512], f32, tag="sc")
                    for hh in (0, 1):
                        prow = slice(64 * hh, 64 * hh + 64)
                        nc.tensor.matmul(
                            ps2[:, hh, :w],
                            lhsT=kT[prow, pair, ts(c, 128)],
                            rhs=qT[prow, pair, ds(512 * qb + q0, w)],
                            start=True,
                            stop=True,
                        )
                    pt = ptp.tile([128, 2, 512], bf16, tag="pt")
                    nc.scalar.activation(pt[:, :, :w], ps2[:, :, :w], Exp)
                    if dc >= 0:
                        # zero the masked upper triangle of the diagonal
                        # 128x128 tile on the (otherwise idle) gpsimd
                        # engine; the lag-2 PV emission below gives it ~2
                        # chunk-slots of slack off the critical path
                        nc.gpsimd.tensor_mul(
                            pt[:, :, 0:128],
                            pt[:, :, 0:128],
                            mask[:, None, :].to_broadcast((128, 2, 128)),
                        )
                    pend.append((pt, c, q0, w))
                    if len(pend) > 3:
                        emit_pv(pend.pop(0), last=False)
                while pend:
                    ent = pend.pop(0)
                    emit_pv(ent, last=(not pend))

                # normalization, returned as a closure the caller defers
                # into the NEXT block (emitted at its chunk 1): one fused
                # [65,512] copy per head frees the PSUM accumulators early,
                # the PE broadcasts the rowsums, DVE takes the reciprocals,
                # and the (PSUM-free) multiplies run on gpsimd
                def do_norm():
                    oUA = oup.tile([65, 512], f32r, tag="ou")
                    oUB = oup.tile([65, 512], f32r, tag="ou")
                    nc.vector.tensor_copy(oUA[:, :], psA[0:65, :])
                    nc.vector.tensor_copy(oUB[:, :], psB[0:65, :])
                    psR = mm.tile([128, 512], f32, tag="mm")
                    nc.tensor.matmul(
                        psR[0:64, :],
                        lhsT=ones[64:65, 0:64],
                        rhs=oUA[64:65, :],
                        start=True,
                        stop=True,
                    )
                    psR2 = mm.tile([128, 512], f32, tag="mm")
                    nc.tensor.matmul(
                        psR2[0:64, :],
                        lhsT=ones[64:65, 0:64],
                        rhs=oUB[64:65, :],
                        start=True,
                        stop=True,
                    )
                    rbA = rbp.tile([64, 512], f32, tag="rb")
                    rbB = rbp.tile([64, 512], f32, tag="rb")
                    nc.vector.reciprocal_approx_fast(rbA[:, :], psR[0:64, :])
                    nc.vector.reciprocal_approx_fast(rbB[:, :], psR2[0:64, :])
                    tmpB = tbp.tile([64, 512], bf16, tag="tb")
                    nc.gpsimd.tensor_mul(
                        oT[0:64, pair, ts(qb, 512)], oUA[0:64, :], rbA[:, :]
                    )
                    nc.gpsimd.tensor_mul(tmpB[:, :], oUB[0:64, :], rbB[:, :])
                    if pair == 1 and qb == 3:
                        # last block: out_proj_B contracts tmpB directly
                        # (via woT2) -- no partition-shift DMA on the tail
                        state["tmpB3"] = tmpB
                    else:
                        nc.sync.dma_start(
                            oT[64:128, pair, ts(qb, 512)], tmpB[:, :]
                        )

                return do_norm

            def out_proj_jt(jt, sb):
                    ps = mm.tile([128, 512], f32, tag="mm")
                    for dchunk in range(2):
                        nc.tensor.matmul(
                            ps,
                            lhsT=woT[:, dchunk, ts(jt, 128)],
                            rhs=oT[:, dchunk, ts(sb, 512)],
                            start=(dchunk == 0),
                            stop=(dchunk == 1),
                        )
                    stg = stp.tile([128, 512], bf16, tag="st", bufs=6)
                    nc.vector.tensor_tensor(
                        stg[:],
                        ps,
                        bo[:, jt : jt + 1].to_broadcast((128, 512)),
                        ADD,
                    )
                    nc.sync.dma_start(out_d.ap()[ts(jt, 128), ts(sb, 512)], stg[:])

            def out_proj_A_jt(jt):
                # pair-0 half of the sb=3 output projection (+ bias); runs
                # as filler inside the final attention block
                ps = mm.tile([128, 512], f32, tag="mm")
                nc.tensor.matmul(
                    ps,
                    lhsT=woT[:, 0, ts(jt, 128)],
                    rhs=oT[:, 0, ts(3, 512)],
                    start=True,
                    stop=True,
                )
                stg = stp.tile([128, 512], bf16, tag="st", bufs=6)
                nc.vector.tensor_tensor(
                    stg[:],
                    ps,
                    bo[:, jt : jt + 1].to_broadcast((128, 512)),
                    ADD,
                )
                nc.sync.dma_start(out_d.ap()[ts(jt, 128), ts(3, 512)], stg[:])

            def out_proj_B_jt(jt):
                # pair-1 half, shipped as a separate bf16 partial (host
                # adds); head-3 dims contracted straight out of tmpB3 (no
                # partition-shift DMA); PSUM->SBUF copy on the tail-idle
                # ACT engine
                ps = mm.tile([128, 512], f32, tag="mm")
                nc.tensor.matmul(
                    ps,
                    lhsT=woT[0:64, 1, ts(jt, 128)],
                    rhs=oT[0:64, 1, ts(3, 512)],
                    start=True,
                    stop=False,
                )
                nc.tensor.matmul(
                    ps,
                    lhsT=woT2[:, ts(jt, 128)],
                    rhs=state["tmpB3"][:, :],
                    start=False,
                    stop=True,
                )
                stg = stp.tile([128, 512], bf16, tag="st2", bufs=4)
                nc.scalar.activation(stg[:], ps, Ident)
                eng = nc.sync if jt % 2 == 0 else nc.scalar
                eng.dma_start(out2_d.ap()[ts(jt, 128), :], stg[:])

            # software-pipelined emission: per q-block wave, produce the
            # projections it needs, then attention, then the output slice
            def emit_A(qb):
                for t in range(2):
                    proj_qk(wqT, qT, t, qb, bias=bq)
                for st in range(4 * qb, 4 * qb + 4):
                    proj_v(st)
                for t in range(2):
                    proj_qk(wkT, kT, t, qb)

            emit_A(0)
            for qb in range(4):
                ath = []
                if qb < 3:
                    nxt = qb + 1
                    for t in range(2):
                        ath.append(
                            lambda t=t, nxt=nxt: proj_qk(wqT, qT, t, nxt, bias=bq)
                        )
                    for st in range(4 * nxt, 4 * nxt + 4):
                        ath.append(lambda st=st: proj_v(st))
                    for t in range(2):
                        ath.append(
                            lambda t=t, nxt=nxt: proj_qk(wkT, kT, t, nxt)
                        )
                cth = []
                if qb == 1:
                    cth = [
                        lambda jt=jt: out_proj_jt(jt, 0) for jt in range(8)
                    ]
                elif qb == 2:
                    cth = [
                        lambda jt=jt: out_proj_jt(jt, 1) for jt in range(4)
                    ]
                elif qb == 3:
                    cth = [
                        lambda jt=jt: out_proj_jt(jt + 4, 1) for jt in range(4)
                    ] + [
                        lambda jt=jt: out_proj_jt(jt, 2) for jt in range(8)
                    ]
                # projections (dependency-free) fill first; out-projections
                # need the previous wave's deferred normalize, so they go
                # later in the wave
                thunks = ath + cth
                pn = state.get("prev_norm")
                pre = [pn] if pn else None
                if qb < 3:
                    fe = max(1, (2 * (4 * qb + 4)) // (len(thunks) + 1))
                    n0 = attn_block(
                        0, qb, fill=thunks, fill_every=fe, prefill=pre
                    )
                    n1 = attn_block(
                        1, qb, fill=thunks, fill_every=fe, prefill=[n0]
                    )
                    state["prev_norm"] = n1
                    for th in thunks:
                        th()
                else:
                    # final wave: pair-0 block takes the shared thunks;
                    # pair-1 block overlaps the pair-0 half of the sb=3
                    # output projection (fill_offset leaves time for the
                    # deferred pair-0 normalize to land)
                    fe = max(1, (4 * qb + 4 - 2) // (len(thunks) + 1))
                    n0 = attn_block(
                        0, qb, fill=thunks, fill_every=fe,
                        fill_offset=2, prefill=pre,
                    )
                    thB = thunks + [
                        lambda jt=jt: out_proj_A_jt(jt) for jt in range(8)
                    ]
                    feB = max(1, (4 * qb + 4 - 8) // (len(thB) + 1))
                    n1 = attn_block(
                        1, qb, fill=thB, fill_every=feB,
                        fill_offset=8, prefill=[n0],
                    )
                    for th in thB:
                        th()
                    n1()
                    for jt in range(8):
                        out_proj_B_jt(jt)
            if DEBUG:
                nc.sync.dma_start(qT_o.ap(), qT[:])
                nc.sync.dma_start(kT_o.ap(), kT[:])
                nc.sync.dma_start(v4_o.ap(), v4[:])
                nc.sync.dma_start(oT_o.ap(), oT[:])

    nc.compile()
    return nc


def _get_bass():
    if "nc" not in _cache:
        _cache["nc"] = _build_bass()
    return _cache["nc"]


def _shard_inputs(x, Wq, bq, Wk, bk, Wv, bv, Wo, bo):
    import ml_dtypes

    bfl = ml_dtypes.bfloat16
    x = np.asarray(x, dtype=np.float32)
    Wq = np.asarray(Wq, dtype=np.float32)
    Wk = np.asarray(Wk, dtype=np.float32)
    Wv = np.asarray(Wv, dtype=np.float32)
    Wo = np.asarray(Wo, dtype=np.float32)
    bq = np.asarray(bq, dtype=np.float32)
    bv = np.asarray(bv, dtype=np.float32)
    bo = np.asarray(bo, dtype=np.float32)

    kk = np.arange(128)[:, None]
    qq = np.arange(128)[None, :]
    mask128 = (kk <= qq).astype(bfl)

    xT = [np.ascontiguousarray(x[b].T).astype(bfl) for b in range(x.shape[0])]

    def prearrange(wT, nchunks):
        # [nchunks*128, F] -> [128, nchunks*F] so each partition's row is
        # one contiguous DMA line
        f = wT.shape[1]
        return np.ascontiguousarray(
            wT.reshape(nchunks, 128, f).transpose(1, 0, 2).reshape(128, -1)
        )
    in_maps = []
    for c in range(NCORES):
        b, g = divmod(c, 4)
        sl = slice(DL * g, DL * (g + 1))
        # bv contributes a constant row to this core's partial output
        # (attention rows sum to 1); bo itself only on the g==0 cores.
        bo_eff = bv[sl] @ Wo[:, sl].T
        if g == 0:
            bo_eff = bo_eff + bo
        in_maps.append(
            {
                "xT": xT[b],
                "wqT": prearrange(Wq[sl].T * 0.125, 8).astype(bfl),
                "wkT": prearrange(Wk[sl].T, 8).astype(bfl),
                "wvT": prearrange(Wv[sl].T, 8).astype(bfl),
                "woT": prearrange(Wo[:, sl].T, 2).astype(bfl),
                "woT2": np.ascontiguousarray(Wo[:, sl].T[192:256]).astype(bfl),
                "bq": np.ascontiguousarray((bq[sl] * 0.125).reshape(2, 128).T),
                "bo": np.ascontiguousarray(bo_eff.reshape(8, 128).T),
                "mask": mask128,
                "onesr": np.ones((128, 64), np.float32),
            }
        )
    return in_maps


def kernel(x, Wq, bq, Wk, bk, Wv, bv, Wo, bo):
    global LAST_EXEC_TIME_NS, LAST_TRACE_PATH
    from concourse.bass_utils import run_bass_kernel_spmd

    nc = _get_bass()
    in_maps = _shard_inputs(x, Wq, bq, Wk, bk, Wv, bv, Wo, bo)

    trace = os.environ.get("KERNEL_TRACE", "0") == "1"
    res = run_bass_kernel_spmd(
        nc, in_maps, core_ids=list(range(NCORES)), trace=trace
    )
    LAST_EXEC_TIME_NS = res.exec_time_ns
    if res.instructions_and_trace is not None:
        LAST_TRACE_PATH = res.instructions_and_trace[1]

    B = 2
    out = np.empty((B, S, D), dtype=np.float32)
    for b in range(B):
        acc = res.results[4 * b]["outT"].astype(np.float32)
        acc[:, 1536:2048] += res.results[4 * b]["outT2"].astype(np.float32)
        for g in range(1, 4):
            acc = acc + res.results[4 * b + g]["outT"].astype(np.float32)
            acc[:, 1536:2048] += res.results[4 * b + g]["outT2"].astype(
                np.float32
            )
        out[b] = acc.T
    return out
